# revision 14
# baseline (speedup 1.0000x reference)
"""Chamfer distance loss kernel for Trainium2 (8 NeuronCores).

Problem: template/source (4, 8192, 3) f32. For each batch b:
  d[n,m] = |t_n|^2 - 2 t_n.s_m + |s_m|^2
  loss_b = mean_n min_m d + mean_m min_n d ; output = mean_b loss_b (scalar).

Sharding: core c handles (batch = c//2, template-row-half = c%2):
4096 template rows x all 8192 source points. The distance matrix tile
[128 rows x 512 cols] is produced directly in PSUM by a single K=5
augmented matmul: lhsT rows = [t0,t1,t2,|t|^2,1], rhs rows =
[-2 s0,-2 s1,-2 s2, 1, |s|^2]. ScalarE evacuates PSUM to SBUF as fp16
(min-selection in fp16 is numerically safe here: ~2.5e-6 final rel err),
VectorE accumulates row-wise and column-wise minima in fp16 (2x packed
mode), and the per-core partials (row-min sums, 16-partition-folded
column minima) are combined on the host.
"""
import os
import sys

sys.path.insert(0, "/opt/trn_rl_repo")

from contextlib import ExitStack

import numpy as np

import concourse.bass as bass
import concourse.tile as tile
from concourse import mybir
from concourse.bass_utils import run_bass_kernel_spmd

# ---------------------------------------------------------------------------
# The walrus build in this container rejects instructions carrying more than
# one sync-wait command. After Tile scheduling, split any multi-wait
# instruction: keep the first wait on it and hoist the rest onto standalone
# EventSemaphore instructions inserted just before it (same engine, so
# per-engine program order makes the waits execute first).
import bass_rust as _br


def split_multi_waits(nc):
    n_new = 0
    for fn in nc.m.functions:
        for blk in fn.blocks:
            insts = list(blk.instructions)
            out = []
            changed = False
            for inst in insts:
                si = inst.sync_info
                waits = list(si.on_wait) if si is not None and si.on_wait else []
                if len(waits) > 1:
                    for w in waits[:-1]:
                        ev = _br.InstEventSemaphore(
                            name=f"I-waitsplit-{n_new}", ins=[], outs=[]
                        )
                        n_new += 1
                        ev.engine = inst.engine
                        ev.sync_info = _br.SyncInfo(on_wait=[w], on_update=[])
                        out.append(ev)
                    si.on_wait = [waits[-1]]
                    changed = True
                out.append(inst)
            if changed:
                blk.instructions = out
# ---------------------------------------------------------------------------

import ml_dtypes

F32 = mybir.dt.float32
F32R = mybir.dt.float32r
F16 = mybir.dt.float16
BF16 = mybir.dt.bfloat16
MIN = mybir.AluOpType.min
BF16NP = ml_dtypes.bfloat16

B, N, M, D = 4, 8192, 8192, 3
R = N // 2      # template rows per core
NCORES = 8
GROUP = 2048    # psum group: 4 matmuls of 512
PFOLD = 16      # colmin partition count returned to host

# "dekker": bf16 3-way-split matmuls, K=24 (1 cycle/row; error at the fp16
#           cast floor ~3.7e-5 final rel err — same as exact-fp32 matmuls)
# "f32"   : exact fp32 matmuls, K=5 (4 cycles/row, slowest, exact)
# "f32r"  : float32r matmuls, K=5 (fast but ~tf32 precision: too coarse)
MM_MODE = os.environ.get("CHAMFER_MM_MODE", "dekker")
K_BY_MODE = {"dekker": 24, "f32": 5, "f32r": 5}
K = K_BY_MODE[MM_MODE]


def build_program(rows=R, cols=M, mm_mode=MM_MODE, split_waits=True):
    row_tiles = rows // 128
    ngroups = cols // GROUP
    k = K_BY_MODE[mm_mode]
    nc = bass.Bass("TRN2", target_bir_lowering=False, debug=False)
    mm_dt = {"dekker": BF16, "f32": F32, "f32r": F32R}[mm_mode]
    lhsT = nc.dram_tensor("lhsT_aug", [k, rows], mm_dt, kind="ExternalInput").ap()
    rhs = nc.dram_tensor("rhs_aug", [k, cols], mm_dt, kind="ExternalInput").ap()
    o_rm = nc.dram_tensor(
        "out_rowmin", [128, row_tiles], F32, kind="ExternalOutput"
    ).ap()
    o_cm = nc.dram_tensor("out_colmin", [PFOLD, cols], F16, kind="ExternalOutput").ap()

    with tile.TileContext(nc) as tc, ExitStack() as ctx:
        consts = ctx.enter_context(tc.tile_pool(name="consts", bufs=1))
        psum_pool = ctx.enter_context(tc.tile_pool(name="psum", bufs=2, space="PSUM"))
        cast_pool = ctx.enter_context(tc.tile_pool(name="cast", bufs=3))
        rowacc_pool = ctx.enter_context(tc.tile_pool(name="rowacc", bufs=2))
        accs = ctx.enter_context(tc.tile_pool(name="accs", bufs=1))

        lhsT_sb = consts.tile([k, rows], mm_dt)
        nc.sync.dma_start(lhsT_sb[:], lhsT)
        rhs_sb = consts.tile([k, cols], mm_dt)
        nc.sync.dma_start(rhs_sb[:], rhs)

        colacc = accs.tile([128, cols], F16)
        rowminb = accs.tile([128, row_tiles], F32)

        for i in range(row_tiles):
            lh = lhsT_sb[:, i * 128:(i + 1) * 128]
            rowacc = rowacc_pool.tile([128, GROUP], F16)
            for g in range(ngroups):
                ps = psum_pool.tile([128, GROUP], F32)
                for jj in range(4):
                    c0 = g * GROUP + jj * 512
                    nc.tensor.matmul(
                        ps[:, jj * 512:(jj + 1) * 512], lh,
                        rhs_sb[:, c0:c0 + 512],
                        start=True, stop=True,
                    )
                if i == 0:
                    dst = colacc[:, g * GROUP:(g + 1) * GROUP]
                    nc.scalar.copy(dst, ps[:])
                    if g == 0:
                        nc.vector.tensor_copy(rowacc[:], dst)
                    else:
                        nc.vector.tensor_tensor(rowacc[:], rowacc[:], dst, op=MIN)
                else:
                    cst = cast_pool.tile([128, GROUP], F16)
                    nc.scalar.copy(cst[:], ps[:])
                    if g == 0:
                        nc.vector.tensor_copy(rowacc[:], cst[:])
                    else:
                        nc.vector.tensor_tensor(rowacc[:], rowacc[:], cst[:], op=MIN)
                    ca = colacc[:, g * GROUP:(g + 1) * GROUP]
                    nc.vector.tensor_tensor(ca, ca, cst[:], op=MIN)
            nc.vector.tensor_reduce(
                rowminb[:, i:i + 1], rowacc[:], axis=mybir.AxisListType.X, op=MIN
            )

        # Fold colacc partitions 128 -> PFOLD. DVE lanes cannot cross
        # partitions, so shift the upper half down via SBUF->SBUF DMA first.
        scratch = accs.tile([64, cols], F16)
        hp = 64
        while hp >= PFOLD:
            nc.sync.dma_start(scratch[0:hp, :], colacc[hp:2 * hp, :])
            nc.vector.tensor_tensor(
                colacc[0:hp, :], colacc[0:hp, :], scratch[0:hp, :], op=MIN
            )
            hp //= 2

        nc.sync.dma_start(o_cm, colacc[0:PFOLD, :])
        nc.sync.dma_start(o_rm, rowminb[:])
    if split_waits:
        split_multi_waits(nc)  # CoreSim can't model the injected waits
    return nc


_program_cache = {}


def _get_program():
    key = (R, M, MM_MODE)
    if key not in _program_cache:
        _program_cache[key] = build_program()
    return _program_cache[key]


def _aug_f32(t, s):
    """K=5 fp32 augmentation: d = |t|^2 - 2 t.s + |s|^2 in one matmul."""
    rows, cols = t.shape[0], s.shape[0]
    lhsT = np.empty((5, rows), np.float32)
    lhsT[0:3] = t.T
    lhsT[3] = (t * t).sum(axis=1)
    lhsT[4] = 1.0
    rhs = np.empty((5, cols), np.float32)
    rhs[0:3] = -2.0 * s.T
    rhs[3] = 1.0
    rhs[4] = (s * s).sum(axis=1)
    return lhsT, rhs


def _split3(x):
    x1 = x.astype(BF16NP)
    r = x - x1.astype(np.float32)
    x2 = r.astype(BF16NP)
    x3 = (r - x2.astype(np.float32)).astype(BF16NP)
    return x1, x2, x3


def _aug_dekker(t, s):
    """K=24 bf16 3-way-split augmentation. Each fp32 value a = a1+a2+a3 in
    bf16 parts; products kept to O(2^-27): a1b1, a1b2, a2b1, a1b3, a3b1,
    a2b2. PE cost is free-dim cycles only, so K=24 runs as fast as K=5."""
    rows, cols = t.shape[0], s.shape[0]
    t1, t2, t3 = _split3(t)
    s1, s2, s3 = _split3(-2.0 * s)
    n1, n2, n3 = _split3((t * t).sum(axis=1))
    m1, m2, m3 = _split3((s * s).sum(axis=1))
    one = np.ones((), BF16NP)
    lhsT = np.empty((24, rows), BF16NP)
    for j, part in enumerate((t1, t1, t2, t1, t3, t2)):
        lhsT[3 * j:3 * j + 3] = part.T
    lhsT[18] = n1
    lhsT[19] = n2
    lhsT[20] = n3
    lhsT[21:24] = one
    rhs = np.empty((24, cols), BF16NP)
    for j, part in enumerate((s1, s2, s1, s3, s1, s2)):
        rhs[3 * j:3 * j + 3] = part.T
    rhs[18:21] = one
    rhs[21] = m1
    rhs[22] = m2
    rhs[23] = m3
    return lhsT, rhs


def make_in_maps(template, source, mm_mode=MM_MODE):
    template = np.asarray(template, dtype=np.float32)
    source = np.asarray(source, dtype=np.float32)
    aug = _aug_dekker if mm_mode == "dekker" else _aug_f32
    in_maps = []
    for c in range(NCORES):
        b, h = c // 2, c % 2
        t = template[b, h * R:(h + 1) * R]      # [R, 3]
        s = source[b]                            # [M, 3]
        lhsT, rhs = aug(t, s)
        in_maps.append(
            {"lhsT_aug": np.ascontiguousarray(lhsT),
             "rhs_aug": np.ascontiguousarray(rhs)}
        )
    return in_maps


last_results = None  # BassKernelResults of the most recent kernel() call


def kernel(template, source):
    global last_results
    nc = _get_program()
    in_maps = make_in_maps(template, source)
    res = run_bass_kernel_spmd(nc, in_maps, list(range(NCORES)))
    last_results = res

    per_batch = np.zeros(B, dtype=np.float64)
    for b in range(B):
        r0 = res.results[2 * b + 0]
        r1 = res.results[2 * b + 1]
        rowsum = (
            r0["out_rowmin"].astype(np.float64).sum()
            + r1["out_rowmin"].astype(np.float64).sum()
        )
        cost_p0_p1 = rowsum / N
        cm = np.minimum(
            r0["out_colmin"].astype(np.float32).min(axis=0),
            r1["out_colmin"].astype(np.float32).min(axis=0),
        )
        cost_p1_p0 = cm.astype(np.float64).mean()
        per_batch[b] = cost_p0_p1 + cost_p1_p0
    return np.float32(per_batch.mean())
